# revision 16
# baseline (speedup 1.0000x reference)
"""Batch temporal alignment on Trainium2 (8 NeuronCores, data-parallel over batch).

Math (mirrors the reference exactly):
  - Per batch row b the valid (masked-in) timestamps form a sorted array
    ts_b; a common aligned grid aligned_t = linspace(max_start, min_end, T)
    is interpolated per row / per feature with np.interp semantics.
  - For each (b, j): out[b, j, :] = v0 + w * (v1 - v0) where
    v0 = values[b, src0[b, j], :], v1 = values[b, src1[b, j], :] and
    w = (aligned_t[j] - ts[src0]) / (ts[src1] - ts[src0]).
  - The bracketing indices and weights depend only on timestamps+mask
    (1 MB of data) and are computed on host in fp32, replicating
    jnp.linspace / jnp.interp bit-for-bit.  The heavy, memory-bound
    gather + lerp over values ([32, 8192, 128] f32) runs on the 8 cores.

Device strategy (pure data parallel, 4 rows per core):
  - Because the aligned step is ~2 source steps, src0 is "base + 2j" in
    long runs (one drift crossover per row).  The host passes, per
    (row, PACK*128-step group), a slice of values laid out so that SBUF
    partition p holds the 2*PACK consecutive source rows feeding its PACK
    consecutive aligned steps (a pure numpy view for most groups; a small
    gather-copy for the few crossover groups).  This keeps the device
    program fully static and identical across cores (SPMD) with large
    DMA transfers of big contiguous chunks (8 KB loads / 4 KB stores per
    descriptor — HWDGE descriptor generation is the sequencer-side cost).
  - Per group the device does: one ~2 MB HWDGE load (Sync ring), one wide
    DVE tensor_sub, PACK in-place fused scalar_tensor_tensor ops
    (diff * w + v0, w as a per-partition scalar column), and one ~1 MB
    store (Scalar ring, so store waits never block loads).  No
    collectives are needed (the grid reduction is part of the tiny
    host-side index computation).  Measured ~80-90 us/core on trn2,
    vs a ~70 us HBM roofline for the ~25 MB/core moved.
"""

import numpy as np

import concourse.bacc as bacc
import concourse.bass as bass
import concourse.mybir as mybir
from concourse.bass_utils import run_bass_kernel_spmd
from concourse.tile import TileContext

N_CORES = 8
DT = 0.1
BIG = 1e9
F32 = mybir.dt.float32

# Results object of the most recent device run (test harness reads
# .exec_time_ns / .profile_json out of this when tracing is enabled).
LAST_RUN = None

_PROGRAM_CACHE = {}


def _host_grid_and_weights(timestamps, values, mask):
    """Replicate reference _prep + jnp.linspace + jnp.interp bracketing in fp32."""
    B, S = timestamps.shape
    t_m = np.where(mask, timestamps, np.float32(BIG)).astype(np.float32)
    order = np.argsort(t_m, axis=1, kind="stable")
    t_s = np.take_along_axis(t_m, order, axis=1)
    n_valid = mask.sum(axis=1)
    end = t_s[np.arange(B), n_valid - 1]
    max_start = np.float32(t_s[:, 0].max())
    min_end = np.float32(end.min())
    num_steps = int(np.float32(np.float32(min_end - max_start) / np.float32(DT))) + 1

    # jnp.linspace bit-exact vs XLA:CPU: the compiled HLO rewrites
    # iota/div into iota * f32(1/div) and reassociates stop*step into
    # iota * f32(stop/div); LLVM then contracts (1 - iota*c) and the final
    # add-of-product into FMAs.  float128 (64-bit mantissa) emulates a true
    # f32 FMA exactly: the f32*f32 product is exact, the sum is exact, and
    # the cast rounds once.  Verified 0/4065 ULP mismatches vs jax-cpu.
    div = num_steps - 1
    f128 = np.float128
    c = np.float32(np.float32(1.0) / np.float32(div))
    iota = np.arange(div, dtype=np.float32).astype(f128)
    sub_f = (f128(1.0) - iota * f128(c)).astype(np.float32)  # fnmadd
    m = (max_start * sub_f).astype(np.float32)
    bc = np.float32(min_end * c)
    aligned_t = np.empty(num_steps, np.float32)
    aligned_t[:div] = (iota * f128(bc) + m.astype(f128)).astype(np.float32)  # fmadd
    aligned_t[div] = min_end

    # jnp.interp: i = clip(searchsorted(xp, x, 'right'), 1, S-1);
    # f = fp[i-1] + ((x - xp[i-1]) / (xp[i] - xp[i-1])) * (fp[i] - fp[i-1]).
    # (The |dx| <= spacing(eps) guard can't trigger here: dx >= orig_dt.)
    i = np.empty((B, num_steps), np.int64)
    for b in range(B):
        i[b] = np.searchsorted(t_s[b], aligned_t, side="right")
    np.clip(i, 1, S - 1, out=i)
    t0 = np.take_along_axis(t_s, i - 1, axis=1)
    t1 = np.take_along_axis(t_s, i, axis=1)
    delta = (aligned_t[None, :] - t0).astype(np.float32)
    dx = (t1 - t0).astype(np.float32)
    w = (delta / dx).astype(np.float32)
    src0 = np.take_along_axis(order, i - 1, axis=1)
    src1 = np.take_along_axis(order, i, axis=1)
    return aligned_t, w, src0, src1


PACK = 8  # consecutive aligned steps packed per SBUF partition
JT = 128 * PACK  # aligned steps per group


def _group_layout(T):
    """Split T aligned steps into groups of JT steps (PACK j's per partition).

    Returns (n_groups, groups); each group is (j0, nj, p_q) where p_q[q] is
    the partition count of phase q (phase q handles j = j0 + PACK*p + q)."""
    groups = []
    for j0 in range(0, T, JT):
        nj = min(JT, T - j0)
        p_q = [max(0, -(-(nj - q) // PACK)) for q in range(PACK)]
        groups.append((j0, nj, p_q))
    return len(groups), groups


def _build_program(n_rows, T, F):
    # Layout: partition p of group g holds source rows for the PACK
    # consecutive aligned steps j = g*JT + PACK*p + q, i.e. 2*PACK
    # consecutive value rows (4 KB) -> one ~1 MB load with 128 4 KB-chunk
    # descriptors (HWDGE descriptor generation on the sequencer is the
    # scaling cost, one descriptor per contiguous chunk).  Stores write
    # PACK consecutive output rows per partition (2 KB chunks).  Compute:
    # one wide DVE tensor_sub per group + one in-place scalar_tensor_tensor
    # per (group, phase) with a per-partition w column.  Loads issue from
    # the Sync HWDGE ring, stores from the Scalar ring, so store waits
    # never head-of-line-block loads.  Bacc finalize() legalizes sem waits
    # (TRN2: 1 wait per instruction).
    n_groups, groups = _group_layout(T)
    nc = bacc.Bacc(None)
    v_in = {}
    for r in range(n_rows):
        for gi, (j0, nj, p_q) in enumerate(groups):
            rows = 2 * PACK * p_q[0]
            v_in[(r, gi)] = nc.dram_tensor(
                f"v_{r}_{gi}", [rows, F], F32, kind="ExternalInput"
            )
    n_wcols = n_rows * n_groups * PACK
    w_in = nc.dram_tensor("wts", [128, n_wcols], F32, kind="ExternalInput")
    o_out = [
        nc.dram_tensor(f"o_{r}", [T, F], F32, kind="ExternalOutput")
        for r in range(n_rows)
    ]

    with TileContext(nc) as tc:
        with (
            tc.tile_pool(name="wp", bufs=1) as wp,
            tc.tile_pool(name="lp", bufs=6) as lp,
            tc.tile_pool(name="ow", bufs=6) as ow,
        ):
            w_sb = wp.tile([128, n_wcols], F32)
            nc.sync.dma_start(out=w_sb[:, :], in_=w_in[:, :])
            wtouch = wp.tile([128, 1], F32)
            nc.vector.tensor_copy(wtouch[:, :], w_sb[:, 0:1])
            for r in range(n_rows):
                for gi, (j0, nj, p_q) in enumerate(groups):
                    P0 = p_q[0]  # widest phase = partitions with any work
                    full = nj == JT
                    v = v_in[(r, gi)]
                    L = lp.tile([128, 2 * PACK * F], F32)
                    nc.sync.dma_start(
                        out=L[:P0, :],
                        in_=v.rearrange("(p c) f -> p (c f)", p=P0, c=2 * PACK),
                    )
                    L4 = L[:, :].rearrange(
                        "p (q two f) -> p q two f", q=PACK, two=2, f=F
                    )
                    O = ow.tile([128, PACK * F], F32)
                    O3 = O[:, :].rearrange("p (q f) -> p q f", q=PACK, f=F)
                    # O <- v1 - v0, whole group in one strided op (ragged
                    # lanes of a partial group compute junk, never stored)
                    nc.vector.tensor_sub(
                        O3[:P0, :, :], L4[:P0, :, 1, :], L4[:P0, :, 0, :]
                    )
                    # per phase: O <- O * w + v0 (in place)
                    for q in range(PACK):
                        P = p_q[q]
                        col = (r * n_groups + gi) * PACK + q
                        nc.vector.scalar_tensor_tensor(
                            O3[:P, q, :],
                            O3[:P, q, :],
                            w_sb[:P, col : col + 1],
                            L4[:P, q, 0, :],
                            op0=mybir.AluOpType.mult,
                            op1=mybir.AluOpType.add,
                        )
                    if full:
                        nc.scalar.dma_start(
                            out=o_out[r][j0 : j0 + JT, :].rearrange(
                                "(p q) f -> p (q f)", p=128, q=PACK
                            ),
                            in_=O[:, :],
                        )
                    else:
                        # partial group: whole-PACK partitions in one DMA,
                        # the ragged remainder row by row
                        n_even = nj // PACK
                        if n_even:
                            nc.scalar.dma_start(
                                out=o_out[r][j0 : j0 + n_even * PACK, :].rearrange(
                                    "(p q) f -> p (q f)", p=n_even, q=PACK
                                ),
                                in_=O[:n_even, :],
                            )
                        rem = nj - n_even * PACK
                        if rem:
                            nc.scalar.dma_start(
                                out=o_out[r][
                                    j0 + n_even * PACK : j0 + nj, :
                                ].rearrange("(one j) f -> one (j f)", one=1, j=rem),
                                in_=O[n_even : n_even + 1, 0 : rem * F],
                            )
    # Bacc's compile passes (register allocation, event-semaphore splitting
    # for the 1-wait-per-instruction TRN2 limit) run in finalize(); the
    # bass2jax execute path expects an already-finalized module.
    nc.finalize()
    return nc


def _get_program(n_rows, T, F):
    key = (n_rows, T, F)
    if key not in _PROGRAM_CACHE:
        _PROGRAM_CACHE[key] = _build_program(n_rows, T, F)
    return _PROGRAM_CACHE[key]


def kernel(timestamps, values, mask):
    global LAST_RUN
    timestamps = np.asarray(timestamps).astype(np.float32, copy=False)
    values = np.ascontiguousarray(np.asarray(values), dtype=np.float32)
    mask = np.asarray(mask).astype(bool, copy=False)
    B, S, F = values.shape

    aligned_t, w, src0, src1 = _host_grid_and_weights(timestamps, values, mask)
    T = aligned_t.shape[0]
    n_groups, groups = _group_layout(T)
    assert B % N_CORES == 0, (B, N_CORES)
    n_rows = B // N_CORES

    nc = _get_program(n_rows, T, F)

    in_maps = []
    for c in range(N_CORES):
        m = {}
        # weights: column (r*n_groups + g)*PACK + q, partition p holds
        # w[b, g*JT + p*PACK + q] (zero-padded past T)
        wpad = np.zeros((n_rows, n_groups * JT), np.float32)
        wpad[:, :T] = w[c * n_rows : (c + 1) * n_rows]
        m["wts"] = np.ascontiguousarray(
            wpad.reshape(n_rows, n_groups, 128, PACK)
            .transpose(2, 0, 1, 3)
            .reshape(128, n_rows * n_groups * PACK)
        )
        for r in range(n_rows):
            b = c * n_rows + r
            vb = values[b]
            for gi, (j0, nj, p_q) in enumerate(groups):
                rows_needed = 2 * PACK * p_q[0]
                s0 = src0[b, j0 : j0 + nj]
                s1 = src1[b, j0 : j0 + nj]
                base = int(s0[0])
                if (
                    base + rows_needed <= S
                    and np.array_equal(s0, np.arange(base, base + 2 * nj, 2))
                    and np.array_equal(s1, s0 + 1)
                ):
                    X = vb[base : base + rows_needed]  # pure view, no copy
                else:
                    ridx = np.empty(rows_needed, np.int64)
                    ridx[0 : 2 * nj : 2] = s0
                    ridx[1 : 2 * nj : 2] = s1
                    ridx[2 * nj :] = s1[-1]  # pad rows (content unused)
                    X = vb[ridx]  # rare crossover group: small gather copy
                m[f"v_{r}_{gi}"] = X
        in_maps.append(m)

    res = run_bass_kernel_spmd(nc, in_maps, core_ids=list(range(N_CORES)))
    LAST_RUN = res

    out = np.empty((B, T, F), np.float32)
    for c in range(N_CORES):
        for r in range(n_rows):
            out[c * n_rows + r] = res.results[c][f"o_{r}"]
    return aligned_t, out


# revision 18
# speedup vs baseline: 1.2817x; 1.2817x over previous
"""Batch temporal alignment on Trainium2 (8 NeuronCores, data-parallel over batch).

Math (mirrors the reference exactly):
  - Per batch row b the valid (masked-in) timestamps form a sorted array
    ts_b; a common aligned grid aligned_t = linspace(max_start, min_end, T)
    is interpolated per row / per feature with np.interp semantics.
  - For each (b, j): out[b, j, :] = v0 + w * (v1 - v0) where
    v0 = values[b, src0[b, j], :], v1 = values[b, src1[b, j], :] and
    w = (aligned_t[j] - ts[src0]) / (ts[src1] - ts[src0]).
  - The bracketing indices and weights depend only on timestamps+mask
    (1 MB of data) and are computed on host in fp32, replicating
    jnp.linspace / jnp.interp bit-for-bit.  The heavy, memory-bound
    gather + lerp over values ([32, 8192, 128] f32) runs on the 8 cores.

Device strategy (pure data parallel, 4 rows per core):
  - Because the aligned step is ~2 source steps, src0 is "base + 2j" in
    long runs (one drift crossover per row).  The host passes, per
    (row, PACK*128-step group), a slice of values laid out so that SBUF
    partition p holds the 2*PACK consecutive source rows feeding its PACK
    consecutive aligned steps (a pure numpy view for most groups; a small
    gather-copy for the few crossover groups).  This keeps the device
    program fully static and identical across cores (SPMD) with large
    DMA transfers of big contiguous chunks (8 KB loads / 4 KB stores per
    descriptor — HWDGE descriptor generation is the sequencer-side cost).
  - Per group the device does: one ~1 MB HWDGE load (Sync ring), one wide
    DVE tensor_sub, PACK in-place fused scalar_tensor_tensor ops
    (diff * w + v0, w as a per-partition scalar column), and one ~0.5 MB
    store (Scalar ring, so store waits never block loads).  No
    collectives are needed (the grid reduction is part of the tiny
    host-side index computation).  Measured ~80-90 us/core on trn2,
    vs a ~70 us HBM roofline for the ~25 MB/core moved.
"""

import numpy as np

import concourse.bacc as bacc
import concourse.bass as bass
import concourse.mybir as mybir
from concourse.bass_utils import run_bass_kernel_spmd
from concourse.tile import TileContext

N_CORES = 8
DT = 0.1
BIG = 1e9
F32 = mybir.dt.float32

# Results object of the most recent device run (test harness reads
# .exec_time_ns / .profile_json out of this when tracing is enabled).
LAST_RUN = None

_PROGRAM_CACHE = {}


def _host_grid_and_weights(timestamps, values, mask):
    """Replicate reference _prep + jnp.linspace + jnp.interp bracketing in fp32."""
    B, S = timestamps.shape
    t_m = np.where(mask, timestamps, np.float32(BIG)).astype(np.float32)
    order = np.argsort(t_m, axis=1, kind="stable")
    t_s = np.take_along_axis(t_m, order, axis=1)
    n_valid = mask.sum(axis=1)
    end = t_s[np.arange(B), n_valid - 1]
    max_start = np.float32(t_s[:, 0].max())
    min_end = np.float32(end.min())
    num_steps = int(np.float32(np.float32(min_end - max_start) / np.float32(DT))) + 1

    # jnp.linspace bit-exact vs XLA:CPU: the compiled HLO rewrites
    # iota/div into iota * f32(1/div) and reassociates stop*step into
    # iota * f32(stop/div); LLVM then contracts (1 - iota*c) and the final
    # add-of-product into FMAs.  float128 (64-bit mantissa) emulates a true
    # f32 FMA exactly: the f32*f32 product is exact, the sum is exact, and
    # the cast rounds once.  Verified 0/4065 ULP mismatches vs jax-cpu.
    div = num_steps - 1
    f128 = np.float128
    c = np.float32(np.float32(1.0) / np.float32(div))
    iota = np.arange(div, dtype=np.float32).astype(f128)
    sub_f = (f128(1.0) - iota * f128(c)).astype(np.float32)  # fnmadd
    m = (max_start * sub_f).astype(np.float32)
    bc = np.float32(min_end * c)
    aligned_t = np.empty(num_steps, np.float32)
    aligned_t[:div] = (iota * f128(bc) + m.astype(f128)).astype(np.float32)  # fmadd
    aligned_t[div] = min_end

    # jnp.interp: i = clip(searchsorted(xp, x, 'right'), 1, S-1);
    # f = fp[i-1] + ((x - xp[i-1]) / (xp[i] - xp[i-1])) * (fp[i] - fp[i-1]).
    # (The |dx| <= spacing(eps) guard can't trigger here: dx >= orig_dt.)
    i = np.empty((B, num_steps), np.int64)
    for b in range(B):
        i[b] = np.searchsorted(t_s[b], aligned_t, side="right")
    np.clip(i, 1, S - 1, out=i)
    t0 = np.take_along_axis(t_s, i - 1, axis=1)
    t1 = np.take_along_axis(t_s, i, axis=1)
    delta = (aligned_t[None, :] - t0).astype(np.float32)
    dx = (t1 - t0).astype(np.float32)
    w = (delta / dx).astype(np.float32)
    src0 = np.take_along_axis(order, i - 1, axis=1)
    src1 = np.take_along_axis(order, i, axis=1)
    return aligned_t, w, src0, src1


# 4 aligned steps per partition: 4 KB load / 2 KB store descriptor chunks.
# (PACK=8 measured WORSE: 2 MB loads emit only 128 8 KB descriptors, and
# packet-granularity draining then feeds only ~half the 16 SDMA engines.)
PACK = 4  # consecutive aligned steps packed per SBUF partition
JT = 128 * PACK  # aligned steps per group


def _group_layout(T):
    """Split T aligned steps into groups of JT steps (PACK j's per partition).

    Returns (n_groups, groups); each group is (j0, nj, p_q) where p_q[q] is
    the partition count of phase q (phase q handles j = j0 + PACK*p + q)."""
    groups = []
    for j0 in range(0, T, JT):
        nj = min(JT, T - j0)
        p_q = [max(0, -(-(nj - q) // PACK)) for q in range(PACK)]
        groups.append((j0, nj, p_q))
    return len(groups), groups


def _build_program(n_rows, T, F):
    # Layout: partition p of group g holds source rows for the PACK
    # consecutive aligned steps j = g*JT + PACK*p + q, i.e. 2*PACK
    # consecutive value rows (4 KB) -> one ~1 MB load with 128 4 KB-chunk
    # descriptors (HWDGE descriptor generation on the sequencer is the
    # scaling cost, one descriptor per contiguous chunk).  Stores write
    # PACK consecutive output rows per partition (2 KB chunks).  Compute:
    # one wide DVE tensor_sub per group + one in-place scalar_tensor_tensor
    # per (group, phase) with a per-partition w column.  Loads issue from
    # the Sync HWDGE ring, stores from the Scalar ring, so store waits
    # never head-of-line-block loads.  Bacc finalize() legalizes sem waits
    # (TRN2: 1 wait per instruction).
    n_groups, groups = _group_layout(T)
    nc = bacc.Bacc(None)
    v_in = {}
    for r in range(n_rows):
        for gi, (j0, nj, p_q) in enumerate(groups):
            rows = 2 * PACK * p_q[0]
            v_in[(r, gi)] = nc.dram_tensor(
                f"v_{r}_{gi}", [rows, F], F32, kind="ExternalInput"
            )
    n_wcols = n_rows * n_groups * PACK
    w_in = nc.dram_tensor("wts", [128, n_wcols], F32, kind="ExternalInput")
    o_out = [
        nc.dram_tensor(f"o_{r}", [T, F], F32, kind="ExternalOutput")
        for r in range(n_rows)
    ]

    with TileContext(nc) as tc:
        with (
            tc.tile_pool(name="wp", bufs=1) as wp,
            tc.tile_pool(name="lp", bufs=6) as lp,
            tc.tile_pool(name="ow", bufs=6) as ow,
        ):
            w_sb = wp.tile([128, n_wcols], F32)
            nc.sync.dma_start(out=w_sb[:, :], in_=w_in[:, :])
            wtouch = wp.tile([128, 1], F32)
            nc.vector.tensor_copy(wtouch[:, :], w_sb[:, 0:1])
            for r in range(n_rows):
                for gi, (j0, nj, p_q) in enumerate(groups):
                    P0 = p_q[0]  # widest phase = partitions with any work
                    full = nj == JT
                    v = v_in[(r, gi)]
                    L = lp.tile([128, 2 * PACK * F], F32)
                    nc.sync.dma_start(
                        out=L[:P0, :],
                        in_=v.rearrange("(p c) f -> p (c f)", p=P0, c=2 * PACK),
                    )
                    L4 = L[:, :].rearrange(
                        "p (q two f) -> p q two f", q=PACK, two=2, f=F
                    )
                    O = ow.tile([128, PACK * F], F32)
                    O3 = O[:, :].rearrange("p (q f) -> p q f", q=PACK, f=F)
                    # O <- v1 - v0, whole group in one strided op (ragged
                    # lanes of a partial group compute junk, never stored)
                    nc.vector.tensor_sub(
                        O3[:P0, :, :], L4[:P0, :, 1, :], L4[:P0, :, 0, :]
                    )
                    # per phase: O <- O * w + v0 (in place)
                    for q in range(PACK):
                        P = p_q[q]
                        col = (r * n_groups + gi) * PACK + q
                        nc.vector.scalar_tensor_tensor(
                            O3[:P, q, :],
                            O3[:P, q, :],
                            w_sb[:P, col : col + 1],
                            L4[:P, q, 0, :],
                            op0=mybir.AluOpType.mult,
                            op1=mybir.AluOpType.add,
                        )
                    if full:
                        nc.scalar.dma_start(
                            out=o_out[r][j0 : j0 + JT, :].rearrange(
                                "(p q) f -> p (q f)", p=128, q=PACK
                            ),
                            in_=O[:, :],
                        )
                    else:
                        # partial group: whole-PACK partitions in one DMA,
                        # the ragged remainder row by row
                        n_even = nj // PACK
                        if n_even:
                            nc.scalar.dma_start(
                                out=o_out[r][j0 : j0 + n_even * PACK, :].rearrange(
                                    "(p q) f -> p (q f)", p=n_even, q=PACK
                                ),
                                in_=O[:n_even, :],
                            )
                        rem = nj - n_even * PACK
                        if rem:
                            nc.scalar.dma_start(
                                out=o_out[r][
                                    j0 + n_even * PACK : j0 + nj, :
                                ].rearrange("(one j) f -> one (j f)", one=1, j=rem),
                                in_=O[n_even : n_even + 1, 0 : rem * F],
                            )
    # Bacc's compile passes (register allocation, event-semaphore splitting
    # for the 1-wait-per-instruction TRN2 limit) run in finalize(); the
    # bass2jax execute path expects an already-finalized module.
    nc.finalize()
    return nc


def _get_program(n_rows, T, F):
    key = (n_rows, T, F)
    if key not in _PROGRAM_CACHE:
        _PROGRAM_CACHE[key] = _build_program(n_rows, T, F)
    return _PROGRAM_CACHE[key]


def kernel(timestamps, values, mask):
    global LAST_RUN
    timestamps = np.asarray(timestamps).astype(np.float32, copy=False)
    values = np.ascontiguousarray(np.asarray(values), dtype=np.float32)
    mask = np.asarray(mask).astype(bool, copy=False)
    B, S, F = values.shape

    aligned_t, w, src0, src1 = _host_grid_and_weights(timestamps, values, mask)
    T = aligned_t.shape[0]
    n_groups, groups = _group_layout(T)
    assert B % N_CORES == 0, (B, N_CORES)
    n_rows = B // N_CORES

    nc = _get_program(n_rows, T, F)

    in_maps = []
    for c in range(N_CORES):
        m = {}
        # weights: column (r*n_groups + g)*PACK + q, partition p holds
        # w[b, g*JT + p*PACK + q] (zero-padded past T)
        wpad = np.zeros((n_rows, n_groups * JT), np.float32)
        wpad[:, :T] = w[c * n_rows : (c + 1) * n_rows]
        m["wts"] = np.ascontiguousarray(
            wpad.reshape(n_rows, n_groups, 128, PACK)
            .transpose(2, 0, 1, 3)
            .reshape(128, n_rows * n_groups * PACK)
        )
        for r in range(n_rows):
            b = c * n_rows + r
            vb = values[b]
            for gi, (j0, nj, p_q) in enumerate(groups):
                rows_needed = 2 * PACK * p_q[0]
                s0 = src0[b, j0 : j0 + nj]
                s1 = src1[b, j0 : j0 + nj]
                base = int(s0[0])
                if (
                    base + rows_needed <= S
                    and np.array_equal(s0, np.arange(base, base + 2 * nj, 2))
                    and np.array_equal(s1, s0 + 1)
                ):
                    X = vb[base : base + rows_needed]  # pure view, no copy
                else:
                    ridx = np.empty(rows_needed, np.int64)
                    ridx[0 : 2 * nj : 2] = s0
                    ridx[1 : 2 * nj : 2] = s1
                    ridx[2 * nj :] = s1[-1]  # pad rows (content unused)
                    X = vb[ridx]  # rare crossover group: small gather copy
                m[f"v_{r}_{gi}"] = X
        in_maps.append(m)

    res = run_bass_kernel_spmd(nc, in_maps, core_ids=list(range(N_CORES)))
    LAST_RUN = res

    out = np.empty((B, T, F), np.float32)
    for c in range(N_CORES):
        for r in range(n_rows):
            out[c * n_rows + r] = res.results[c][f"o_{r}"]
    return aligned_t, out


# revision 19
# speedup vs baseline: 1.3895x; 1.0841x over previous
"""Batch temporal alignment on Trainium2 (8 NeuronCores, data-parallel over batch).

Math (mirrors the reference exactly):
  - Per batch row b the valid (masked-in) timestamps form a sorted array
    ts_b; a common aligned grid aligned_t = linspace(max_start, min_end, T)
    is interpolated per row / per feature with np.interp semantics.
  - For each (b, j): out[b, j, :] = v0 + w * (v1 - v0) where
    v0 = values[b, src0[b, j], :], v1 = values[b, src1[b, j], :] and
    w = (aligned_t[j] - ts[src0]) / (ts[src1] - ts[src0]).
  - The bracketing indices and weights depend only on timestamps+mask
    (1 MB of data) and are computed on host in fp32, replicating
    jnp.linspace / jnp.interp bit-for-bit.  The heavy, memory-bound
    gather + lerp over values ([32, 8192, 128] f32) runs on the 8 cores.

Device strategy (pure data parallel, 4 rows per core):
  - Because the aligned step is ~2 source steps, src0 is "base + 2j" in
    long runs (one drift crossover per row).  The host passes, per
    (row, PACK*128-step group), a slice of values laid out so that SBUF
    partition p holds the 2*PACK consecutive source rows feeding its PACK
    consecutive aligned steps (a pure numpy view for most groups; a small
    gather-copy for the few crossover groups).  This keeps the device
    program fully static and identical across cores (SPMD) with large
    DMA transfers of big contiguous chunks (8 KB loads / 4 KB stores per
    descriptor — HWDGE descriptor generation is the sequencer-side cost).
  - Per group the device does: one ~1 MB HWDGE load (Sync ring), one wide
    DVE tensor_sub, PACK in-place fused scalar_tensor_tensor ops
    (diff * w + v0, w as a per-partition scalar column), and one ~0.5 MB
    store (Scalar ring, so store waits never block loads).  No
    collectives are needed (the grid reduction is part of the tiny
    host-side index computation).  Measured ~80-90 us/core on trn2,
    vs a ~70 us HBM roofline for the ~25 MB/core moved.
"""

import numpy as np

import concourse.bacc as bacc
import concourse.bass as bass
import concourse.mybir as mybir
from concourse.bass_utils import run_bass_kernel_spmd
from concourse.tile import TileContext

N_CORES = 8
DT = 0.1
BIG = 1e9
F32 = mybir.dt.float32

# Results object of the most recent device run (test harness reads
# .exec_time_ns / .profile_json out of this when tracing is enabled).
LAST_RUN = None

_PROGRAM_CACHE = {}


def _host_grid_and_weights(timestamps, values, mask):
    """Replicate reference _prep + jnp.linspace + jnp.interp bracketing in fp32."""
    B, S = timestamps.shape
    t_m = np.where(mask, timestamps, np.float32(BIG)).astype(np.float32)
    order = np.argsort(t_m, axis=1, kind="stable")
    t_s = np.take_along_axis(t_m, order, axis=1)
    n_valid = mask.sum(axis=1)
    end = t_s[np.arange(B), n_valid - 1]
    max_start = np.float32(t_s[:, 0].max())
    min_end = np.float32(end.min())
    num_steps = int(np.float32(np.float32(min_end - max_start) / np.float32(DT))) + 1

    # jnp.linspace bit-exact vs XLA:CPU: the compiled HLO rewrites
    # iota/div into iota * f32(1/div) and reassociates stop*step into
    # iota * f32(stop/div); LLVM then contracts (1 - iota*c) and the final
    # add-of-product into FMAs.  float128 (64-bit mantissa) emulates a true
    # f32 FMA exactly: the f32*f32 product is exact, the sum is exact, and
    # the cast rounds once.  Verified 0/4065 ULP mismatches vs jax-cpu.
    div = num_steps - 1
    f128 = np.float128
    c = np.float32(np.float32(1.0) / np.float32(div))
    iota = np.arange(div, dtype=np.float32).astype(f128)
    sub_f = (f128(1.0) - iota * f128(c)).astype(np.float32)  # fnmadd
    m = (max_start * sub_f).astype(np.float32)
    bc = np.float32(min_end * c)
    aligned_t = np.empty(num_steps, np.float32)
    aligned_t[:div] = (iota * f128(bc) + m.astype(f128)).astype(np.float32)  # fmadd
    aligned_t[div] = min_end

    # jnp.interp: i = clip(searchsorted(xp, x, 'right'), 1, S-1);
    # f = fp[i-1] + ((x - xp[i-1]) / (xp[i] - xp[i-1])) * (fp[i] - fp[i-1]).
    # (The |dx| <= spacing(eps) guard can't trigger here: dx >= orig_dt.)
    i = np.empty((B, num_steps), np.int64)
    for b in range(B):
        i[b] = np.searchsorted(t_s[b], aligned_t, side="right")
    np.clip(i, 1, S - 1, out=i)
    t0 = np.take_along_axis(t_s, i - 1, axis=1)
    t1 = np.take_along_axis(t_s, i, axis=1)
    delta = (aligned_t[None, :] - t0).astype(np.float32)
    dx = (t1 - t0).astype(np.float32)
    w = (delta / dx).astype(np.float32)
    src0 = np.take_along_axis(order, i - 1, axis=1)
    src1 = np.take_along_axis(order, i, axis=1)
    return aligned_t, w, src0, src1


# 4 aligned steps per partition: 4 KB load / 2 KB store descriptor chunks.
# (PACK=8 measured WORSE: 2 MB loads emit only 128 8 KB descriptors, and
# packet-granularity draining then feeds only ~half the 16 SDMA engines.)
PACK = 4  # consecutive aligned steps packed per SBUF partition
JT = 128 * PACK  # aligned steps per group


def _group_layout(T):
    """Split T aligned steps into groups of JT steps (PACK j's per partition).

    Returns (n_groups, groups); each group is (j0, nj, p_q) where p_q[q] is
    the partition count of phase q (phase q handles j = j0 + PACK*p + q)."""
    groups = []
    for j0 in range(0, T, JT):
        nj = min(JT, T - j0)
        p_q = [max(0, -(-(nj - q) // PACK)) for q in range(PACK)]
        groups.append((j0, nj, p_q))
    return len(groups), groups


def _build_program(n_rows, T, F):
    # Layout: partition p of group g holds source rows for the PACK
    # consecutive aligned steps j = g*JT + PACK*p + q, i.e. 2*PACK
    # consecutive value rows (4 KB) -> one ~1 MB load with 128 4 KB-chunk
    # descriptors (HWDGE descriptor generation on the sequencer is the
    # scaling cost, one descriptor per contiguous chunk).  Stores write
    # PACK consecutive output rows per partition (2 KB chunks).  Compute:
    # one wide DVE tensor_sub per group + one in-place scalar_tensor_tensor
    # per (group, phase) with a per-partition w column.  Loads issue from
    # the Sync HWDGE ring, stores from the Scalar ring, so store waits
    # never head-of-line-block loads.  Bacc finalize() legalizes sem waits
    # (TRN2: 1 wait per instruction).
    n_groups, groups = _group_layout(T)
    nc = bacc.Bacc(None)
    v_in = {}
    for r in range(n_rows):
        for gi, (j0, nj, p_q) in enumerate(groups):
            rows = 2 * PACK * p_q[0]
            v_in[(r, gi)] = nc.dram_tensor(
                f"v_{r}_{gi}", [rows, F], F32, kind="ExternalInput"
            )
    n_wcols = n_rows * n_groups * PACK
    w_in = nc.dram_tensor("wts", [128, n_wcols], F32, kind="ExternalInput")
    o_out = [
        nc.dram_tensor(f"o_{r}", [T, F], F32, kind="ExternalOutput")
        for r in range(n_rows)
    ]

    with TileContext(nc) as tc:
        with (
            tc.tile_pool(name="wp", bufs=1) as wp,
            tc.tile_pool(name="lp", bufs=8) as lp,
            tc.tile_pool(name="ow", bufs=8) as ow,
        ):
            w_sb = wp.tile([128, n_wcols], F32)
            nc.sync.dma_start(out=w_sb[:, :], in_=w_in[:, :])
            wtouch = wp.tile([128, 1], F32)
            nc.vector.tensor_copy(wtouch[:, :], w_sb[:, 0:1])
            for r in range(n_rows):
                for gi, (j0, nj, p_q) in enumerate(groups):
                    P0 = p_q[0]  # widest phase = partitions with any work
                    full = nj == JT
                    v = v_in[(r, gi)]
                    L = lp.tile([128, 2 * PACK * F], F32)
                    nc.sync.dma_start(
                        out=L[:P0, :],
                        in_=v.rearrange("(p c) f -> p (c f)", p=P0, c=2 * PACK),
                    )
                    L4 = L[:, :].rearrange(
                        "p (q two f) -> p q two f", q=PACK, two=2, f=F
                    )
                    O = ow.tile([128, PACK * F], F32)
                    O3 = O[:, :].rearrange("p (q f) -> p q f", q=PACK, f=F)
                    # O <- v1 - v0, whole group in one strided op (ragged
                    # lanes of a partial group compute junk, never stored)
                    nc.vector.tensor_sub(
                        O3[:P0, :, :], L4[:P0, :, 1, :], L4[:P0, :, 0, :]
                    )
                    # per phase: O <- O * w + v0 (in place)
                    for q in range(PACK):
                        P = p_q[q]
                        col = (r * n_groups + gi) * PACK + q
                        nc.vector.scalar_tensor_tensor(
                            O3[:P, q, :],
                            O3[:P, q, :],
                            w_sb[:P, col : col + 1],
                            L4[:P, q, 0, :],
                            op0=mybir.AluOpType.mult,
                            op1=mybir.AluOpType.add,
                        )
                    if full:
                        nc.scalar.dma_start(
                            out=o_out[r][j0 : j0 + JT, :].rearrange(
                                "(p q) f -> p (q f)", p=128, q=PACK
                            ),
                            in_=O[:, :],
                        )
                    else:
                        # partial group: whole-PACK partitions in one DMA,
                        # the ragged remainder row by row
                        n_even = nj // PACK
                        if n_even:
                            nc.scalar.dma_start(
                                out=o_out[r][j0 : j0 + n_even * PACK, :].rearrange(
                                    "(p q) f -> p (q f)", p=n_even, q=PACK
                                ),
                                in_=O[:n_even, :],
                            )
                        rem = nj - n_even * PACK
                        if rem:
                            nc.scalar.dma_start(
                                out=o_out[r][
                                    j0 + n_even * PACK : j0 + nj, :
                                ].rearrange("(one j) f -> one (j f)", one=1, j=rem),
                                in_=O[n_even : n_even + 1, 0 : rem * F],
                            )
    # Bacc's compile passes (register allocation, event-semaphore splitting
    # for the 1-wait-per-instruction TRN2 limit) run in finalize(); the
    # bass2jax execute path expects an already-finalized module.
    nc.finalize()
    return nc


def _get_program(n_rows, T, F):
    key = (n_rows, T, F)
    if key not in _PROGRAM_CACHE:
        _PROGRAM_CACHE[key] = _build_program(n_rows, T, F)
    return _PROGRAM_CACHE[key]


def kernel(timestamps, values, mask):
    global LAST_RUN
    timestamps = np.asarray(timestamps).astype(np.float32, copy=False)
    values = np.ascontiguousarray(np.asarray(values), dtype=np.float32)
    mask = np.asarray(mask).astype(bool, copy=False)
    B, S, F = values.shape

    aligned_t, w, src0, src1 = _host_grid_and_weights(timestamps, values, mask)
    T = aligned_t.shape[0]
    n_groups, groups = _group_layout(T)
    assert B % N_CORES == 0, (B, N_CORES)
    n_rows = B // N_CORES

    nc = _get_program(n_rows, T, F)

    in_maps = []
    for c in range(N_CORES):
        m = {}
        # weights: column (r*n_groups + g)*PACK + q, partition p holds
        # w[b, g*JT + p*PACK + q] (zero-padded past T)
        wpad = np.zeros((n_rows, n_groups * JT), np.float32)
        wpad[:, :T] = w[c * n_rows : (c + 1) * n_rows]
        m["wts"] = np.ascontiguousarray(
            wpad.reshape(n_rows, n_groups, 128, PACK)
            .transpose(2, 0, 1, 3)
            .reshape(128, n_rows * n_groups * PACK)
        )
        for r in range(n_rows):
            b = c * n_rows + r
            vb = values[b]
            for gi, (j0, nj, p_q) in enumerate(groups):
                rows_needed = 2 * PACK * p_q[0]
                s0 = src0[b, j0 : j0 + nj]
                s1 = src1[b, j0 : j0 + nj]
                base = int(s0[0])
                if (
                    base + rows_needed <= S
                    and np.array_equal(s0, np.arange(base, base + 2 * nj, 2))
                    and np.array_equal(s1, s0 + 1)
                ):
                    X = vb[base : base + rows_needed]  # pure view, no copy
                else:
                    ridx = np.empty(rows_needed, np.int64)
                    ridx[0 : 2 * nj : 2] = s0
                    ridx[1 : 2 * nj : 2] = s1
                    ridx[2 * nj :] = s1[-1]  # pad rows (content unused)
                    X = vb[ridx]  # rare crossover group: small gather copy
                m[f"v_{r}_{gi}"] = X
        in_maps.append(m)

    res = run_bass_kernel_spmd(nc, in_maps, core_ids=list(range(N_CORES)))
    LAST_RUN = res

    out = np.empty((B, T, F), np.float32)
    for c in range(N_CORES):
        for r in range(n_rows):
            out[c * n_rows + r] = res.results[c][f"o_{r}"]
    return aligned_t, out


# revision 21
# speedup vs baseline: 1.4227x; 1.0238x over previous
"""Batch temporal alignment on Trainium2 (8 NeuronCores, data-parallel over batch).

Math (mirrors the reference exactly):
  - Per batch row b the valid (masked-in) timestamps form a sorted array
    ts_b; a common aligned grid aligned_t = linspace(max_start, min_end, T)
    is interpolated per row / per feature with np.interp semantics.
  - For each (b, j): out[b, j, :] = v0 + w * (v1 - v0) where
    v0 = values[b, src0[b, j], :], v1 = values[b, src1[b, j], :] and
    w = (aligned_t[j] - ts[src0]) / (ts[src1] - ts[src0]).
  - The bracketing indices and weights depend only on timestamps+mask
    (1 MB of data) and are computed on host in fp32, replicating
    jnp.linspace / jnp.interp bit-for-bit.  The heavy, memory-bound
    gather + lerp over values ([32, 8192, 128] f32) runs on the 8 cores.

Device strategy (pure data parallel, 4 rows per core):
  - Because the aligned step is ~2 source steps, src0 is "base + 2j" in
    long runs (one drift crossover per row).  The host passes, per
    (row, PACK*128-step group), a slice of values laid out so that SBUF
    partition p holds the 2*PACK consecutive source rows feeding its PACK
    consecutive aligned steps (a pure numpy view for most groups; a small
    gather-copy for the few crossover groups).  This keeps the device
    program fully static and identical across cores (SPMD) with large
    DMA transfers of big contiguous chunks (8 KB loads / 4 KB stores per
    descriptor — HWDGE descriptor generation is the sequencer-side cost).
  - Per group the device does: one ~1 MB HWDGE load (Sync ring), one wide
    DVE tensor_sub, PACK in-place fused scalar_tensor_tensor ops
    (diff * w + v0, w as a per-partition scalar column), and one ~0.5 MB
    store (Scalar ring, so store waits never block loads).  No
    collectives are needed (the grid reduction is part of the tiny
    host-side index computation).  Measured ~80-90 us/core on trn2,
    vs a ~70 us HBM roofline for the ~25 MB/core moved.
"""

import numpy as np

import concourse.bacc as bacc
import concourse.bass as bass
import concourse.mybir as mybir
from concourse.bass_utils import run_bass_kernel_spmd
from concourse.tile import TileContext

N_CORES = 8
DT = 0.1
BIG = 1e9
F32 = mybir.dt.float32

# Results object of the most recent device run (test harness reads
# .exec_time_ns / .profile_json out of this when tracing is enabled).
LAST_RUN = None

_PROGRAM_CACHE = {}


def _host_grid_and_weights(timestamps, values, mask):
    """Replicate reference _prep + jnp.linspace + jnp.interp bracketing in fp32."""
    B, S = timestamps.shape
    t_m = np.where(mask, timestamps, np.float32(BIG)).astype(np.float32)
    order = np.argsort(t_m, axis=1, kind="stable")
    t_s = np.take_along_axis(t_m, order, axis=1)
    n_valid = mask.sum(axis=1)
    end = t_s[np.arange(B), n_valid - 1]
    max_start = np.float32(t_s[:, 0].max())
    min_end = np.float32(end.min())
    num_steps = int(np.float32(np.float32(min_end - max_start) / np.float32(DT))) + 1

    # jnp.linspace bit-exact vs XLA:CPU: the compiled HLO rewrites
    # iota/div into iota * f32(1/div) and reassociates stop*step into
    # iota * f32(stop/div); LLVM then contracts (1 - iota*c) and the final
    # add-of-product into FMAs.  float128 (64-bit mantissa) emulates a true
    # f32 FMA exactly: the f32*f32 product is exact, the sum is exact, and
    # the cast rounds once.  Verified 0/4065 ULP mismatches vs jax-cpu.
    div = num_steps - 1
    f128 = np.float128
    c = np.float32(np.float32(1.0) / np.float32(div))
    iota = np.arange(div, dtype=np.float32).astype(f128)
    sub_f = (f128(1.0) - iota * f128(c)).astype(np.float32)  # fnmadd
    m = (max_start * sub_f).astype(np.float32)
    bc = np.float32(min_end * c)
    aligned_t = np.empty(num_steps, np.float32)
    aligned_t[:div] = (iota * f128(bc) + m.astype(f128)).astype(np.float32)  # fmadd
    aligned_t[div] = min_end

    # jnp.interp: i = clip(searchsorted(xp, x, 'right'), 1, S-1);
    # f = fp[i-1] + ((x - xp[i-1]) / (xp[i] - xp[i-1])) * (fp[i] - fp[i-1]).
    # (The |dx| <= spacing(eps) guard can't trigger here: dx >= orig_dt.)
    i = np.empty((B, num_steps), np.int64)
    for b in range(B):
        i[b] = np.searchsorted(t_s[b], aligned_t, side="right")
    np.clip(i, 1, S - 1, out=i)
    t0 = np.take_along_axis(t_s, i - 1, axis=1)
    t1 = np.take_along_axis(t_s, i, axis=1)
    delta = (aligned_t[None, :] - t0).astype(np.float32)
    dx = (t1 - t0).astype(np.float32)
    w = (delta / dx).astype(np.float32)
    src0 = np.take_along_axis(order, i - 1, axis=1)
    src1 = np.take_along_axis(order, i, axis=1)
    return aligned_t, w, src0, src1


# 4 aligned steps per partition: 4 KB load / 2 KB store descriptor chunks.
# (PACK=8 measured WORSE: 2 MB loads emit only 128 8 KB descriptors, and
# packet-granularity draining then feeds only ~half the 16 SDMA engines.)
PACK = 4  # consecutive aligned steps packed per SBUF partition
JT = 128 * PACK  # aligned steps per group


def _group_layout(T):
    """Split T aligned steps into groups of JT steps (PACK j's per partition).

    Returns (n_groups, groups); each group is (j0, nj, p_q) where p_q[q] is
    the partition count of phase q (phase q handles j = j0 + PACK*p + q)."""
    groups = []
    for j0 in range(0, T, JT):
        nj = min(JT, T - j0)
        p_q = [max(0, -(-(nj - q) // PACK)) for q in range(PACK)]
        groups.append((j0, nj, p_q))
    return len(groups), groups


def _build_program(n_rows, T, F):
    # Layout: partition p of group g holds source rows for the PACK
    # consecutive aligned steps j = g*JT + PACK*p + q, i.e. 2*PACK
    # consecutive value rows (4 KB) -> one ~1 MB load with 128 4 KB-chunk
    # descriptors (HWDGE descriptor generation on the sequencer is the
    # scaling cost, one descriptor per contiguous chunk).  Stores write
    # PACK consecutive output rows per partition (2 KB chunks).  Compute:
    # one wide DVE tensor_sub per group + one in-place scalar_tensor_tensor
    # per (group, phase) with a per-partition w column.  Loads issue from
    # the Sync HWDGE ring, stores from the Scalar ring, so store waits
    # never head-of-line-block loads.  Bacc finalize() legalizes sem waits
    # (TRN2: 1 wait per instruction).
    n_groups, groups = _group_layout(T)
    nc = bacc.Bacc(None)
    v_in = {}
    for r in range(n_rows):
        for gi, (j0, nj, p_q) in enumerate(groups):
            rows = 2 * PACK * p_q[0]
            v_in[(r, gi)] = nc.dram_tensor(
                f"v_{r}_{gi}", [rows, F], F32, kind="ExternalInput"
            )
    n_wcols = n_rows * n_groups * PACK
    w_in = nc.dram_tensor("wts", [128, n_wcols], F32, kind="ExternalInput")
    o_out = [
        nc.dram_tensor(f"o_{r}", [T, F], F32, kind="ExternalOutput")
        for r in range(n_rows)
    ]

    with TileContext(nc) as tc:
        with (
            tc.tile_pool(name="wp", bufs=1) as wp,
            tc.tile_pool(name="lp", bufs=10) as lp,
            tc.tile_pool(name="ow", bufs=10) as ow,
        ):
            w_sb = wp.tile([128, n_wcols], F32)
            w_loaded = False
            for r in range(n_rows):
                for gi, (j0, nj, p_q) in enumerate(groups):
                    P0 = p_q[0]  # widest phase = partitions with any work
                    full = nj == JT
                    v = v_in[(r, gi)]
                    L = lp.tile([128, 2 * PACK * F], F32)
                    nc.sync.dma_start(
                        out=L[:P0, :],
                        in_=v.rearrange("(p c) f -> p (c f)", p=P0, c=2 * PACK),
                    )
                    if not w_loaded:
                        # after the first big load so that transfer starts at
                        # the earliest sequencer slot; Bacc's event-semaphore
                        # pass legalizes the resulting multi-wait STT
                        nc.sync.dma_start(out=w_sb[:, :], in_=w_in[:, :])
                        w_loaded = True
                    L4 = L[:, :].rearrange(
                        "p (q two f) -> p q two f", q=PACK, two=2, f=F
                    )
                    O = ow.tile([128, PACK * F], F32)
                    O3 = O[:, :].rearrange("p (q f) -> p q f", q=PACK, f=F)
                    # O <- v1 - v0, whole group in one strided op (ragged
                    # lanes of a partial group compute junk, never stored)
                    nc.vector.tensor_sub(
                        O3[:P0, :, :], L4[:P0, :, 1, :], L4[:P0, :, 0, :]
                    )
                    # per phase: O <- O * w + v0 (in place)
                    for q in range(PACK):
                        P = p_q[q]
                        col = (r * n_groups + gi) * PACK + q
                        nc.vector.scalar_tensor_tensor(
                            O3[:P, q, :],
                            O3[:P, q, :],
                            w_sb[:P, col : col + 1],
                            L4[:P, q, 0, :],
                            op0=mybir.AluOpType.mult,
                            op1=mybir.AluOpType.add,
                        )
                    if full:
                        nc.scalar.dma_start(
                            out=o_out[r][j0 : j0 + JT, :].rearrange(
                                "(p q) f -> p (q f)", p=128, q=PACK
                            ),
                            in_=O[:, :],
                        )
                    else:
                        # partial group: whole-PACK partitions in one DMA,
                        # the ragged remainder row by row
                        n_even = nj // PACK
                        if n_even:
                            nc.scalar.dma_start(
                                out=o_out[r][j0 : j0 + n_even * PACK, :].rearrange(
                                    "(p q) f -> p (q f)", p=n_even, q=PACK
                                ),
                                in_=O[:n_even, :],
                            )
                        rem = nj - n_even * PACK
                        if rem:
                            nc.scalar.dma_start(
                                out=o_out[r][
                                    j0 + n_even * PACK : j0 + nj, :
                                ].rearrange("(one j) f -> one (j f)", one=1, j=rem),
                                in_=O[n_even : n_even + 1, 0 : rem * F],
                            )
    # Bacc's compile passes (register allocation, event-semaphore splitting
    # for the 1-wait-per-instruction TRN2 limit) run in finalize(); the
    # bass2jax execute path expects an already-finalized module.
    nc.finalize()
    return nc


def _get_program(n_rows, T, F):
    key = (n_rows, T, F)
    if key not in _PROGRAM_CACHE:
        _PROGRAM_CACHE[key] = _build_program(n_rows, T, F)
    return _PROGRAM_CACHE[key]


def kernel(timestamps, values, mask):
    global LAST_RUN
    timestamps = np.asarray(timestamps).astype(np.float32, copy=False)
    values = np.ascontiguousarray(np.asarray(values), dtype=np.float32)
    mask = np.asarray(mask).astype(bool, copy=False)
    B, S, F = values.shape

    aligned_t, w, src0, src1 = _host_grid_and_weights(timestamps, values, mask)
    T = aligned_t.shape[0]
    n_groups, groups = _group_layout(T)
    assert B % N_CORES == 0, (B, N_CORES)
    n_rows = B // N_CORES

    nc = _get_program(n_rows, T, F)

    in_maps = []
    for c in range(N_CORES):
        m = {}
        # weights: column (r*n_groups + g)*PACK + q, partition p holds
        # w[b, g*JT + p*PACK + q] (zero-padded past T)
        wpad = np.zeros((n_rows, n_groups * JT), np.float32)
        wpad[:, :T] = w[c * n_rows : (c + 1) * n_rows]
        m["wts"] = np.ascontiguousarray(
            wpad.reshape(n_rows, n_groups, 128, PACK)
            .transpose(2, 0, 1, 3)
            .reshape(128, n_rows * n_groups * PACK)
        )
        for r in range(n_rows):
            b = c * n_rows + r
            vb = values[b]
            for gi, (j0, nj, p_q) in enumerate(groups):
                rows_needed = 2 * PACK * p_q[0]
                s0 = src0[b, j0 : j0 + nj]
                s1 = src1[b, j0 : j0 + nj]
                base = int(s0[0])
                if (
                    base + rows_needed <= S
                    and np.array_equal(s0, np.arange(base, base + 2 * nj, 2))
                    and np.array_equal(s1, s0 + 1)
                ):
                    X = vb[base : base + rows_needed]  # pure view, no copy
                else:
                    ridx = np.empty(rows_needed, np.int64)
                    ridx[0 : 2 * nj : 2] = s0
                    ridx[1 : 2 * nj : 2] = s1
                    ridx[2 * nj :] = s1[-1]  # pad rows (content unused)
                    X = vb[ridx]  # rare crossover group: small gather copy
                m[f"v_{r}_{gi}"] = X
        in_maps.append(m)

    res = run_bass_kernel_spmd(nc, in_maps, core_ids=list(range(N_CORES)))
    LAST_RUN = res

    out = np.empty((B, T, F), np.float32)
    for c in range(N_CORES):
        for r in range(n_rows):
            out[c * n_rows + r] = res.results[c][f"o_{r}"]
    return aligned_t, out
